# revision 19
# baseline (speedup 1.0000x reference)
"""Trainium2 Bass kernel for nn_DLI_loss_full.

Key algebraic fact: logits[b,j,k] = hw[b,j] + xw[b,k] and the loss is
sum(lse - tgt) over valid groups, so the hw[b,j] term (the whole LSTM
path) cancels exactly:

    per_group[b,j] = log(sum_{k=j+1}^{len_b-1} exp(xw[b,k])) - xw[b,j+1]
    loss = sum(per_group) / sum_b(len_b - 1)

with xw = encoder_output @ w_fc[HID:].  The kernel only streams
encoder_output once (memory-bound; ~6.3MB/core, ~311 B/ns sustained).

Structure (trace-driven, v3):
  * stream DMAs cast f32->bf16 in the SDMA datapath (SWDGE path; HBM
    reads unchanged, SBUF writes halved) so the multiply runs on DVE in
    bf16 2x perf mode and the 256->1 reduction runs as two bf16 2x
    tree-add halvings plus a 64-wide tensor_reduce.
  * ALL constants (w replicated, the chunk-combine matrix, and the
    host-precomputed mask tensors mf/wm/amask) ride in ONE packed
    tensor issued FIFO-first on the SAME SWDGE queue as the stream: a
    queue that isn't the majority-traffic queue gets starved to
    single-digit B/ns while the stream runs (measured), so sharing the
    stream's queue is the only ordering guarantee.
  * piece sizes are non-uniform (4,10,10,10,10,4 timesteps of the 48
    per chunk): a small first piece starts the DVE pipeline early, a
    small last piece shrinks the after-last-byte compute tail, and few
    big middle pieces amortize the ~0.5us/piece DVE instruction
    overhead.  gpsimd gets NO elementwise work: a gpsimd tensor_tensor
    running concurrently halves DVE 2x-mode throughput (measured).
  * the mask is folded in additively ((mf-1)*30) before a single
    exp-with-accumulate on the scalar engine: the accum IS the chunk
    total, and masked exps are e^-30 (not 0) so every suffix ln stays
    finite with no epsilon pass.
  * a dummy Ln that READS em[0:1] is placed right after the exp: the
    data dependency pins it there (an input-free warm gets hoisted by
    the scheduler into the exp table's residency window, causing a
    table-load ping-pong, also measured), so the Ln table load overlaps
    the matmul/scan chain instead of the critical path.
"""

from contextlib import ExitStack

import numpy as np

import concourse.bacc as bacc
import concourse.mybir as mybir
import concourse.tile as tile
from concourse import bass_utils

B, T, D, HID = 128, 384, 256, 256
NCORES = 8
BS = B // NCORES            # 16 batches per core
CH = 8                      # chunks per sequence
L = T // CH                 # 48 timesteps per chunk
P = BS * CH                 # 128 partitions
SZ = (3, 10, 10, 10, 9, 6)         # timesteps per piece (sum = L)
OFF = tuple(np.cumsum((0,) + SZ)[:len(SZ)])
NP = len(SZ)
MAXK = max(SZ)
F32 = mybir.dt.float32
BF16 = mybir.dt.bfloat16
NEGM = 30.0                 # additive mask depth: exp(xw-30) ~ 1e-13

# w rides alone ahead of the first x piece (it gates the whole DVE
# pipeline); everything else (only needed from mid-kernel on) follows
# the first x piece.  In bf16 columns:
PK_UM = 0                   # chunk-combine matrix         [P, P]      bf16
PK_MF = PK_UM + P           # mask as f32                  [P, L]      f32
PK_WM = PK_MF + 2 * L       # group-validity weights f32   [P, L]      f32
PK_AM = PK_WM + 2 * L       # additive mask (mf-1)*30 f32  [P, L]      f32
PK_N = PK_AM + 2 * L

_cache = {}


def _build_nc():
    nc = bacc.Bacc(
        "TRN2", target_bir_lowering=False, debug=False, num_devices=NCORES
    )
    x = nc.dram_tensor("x", [BS, T, D], F32, kind="ExternalInput").ap()
    pw = nc.dram_tensor("pw", [P, D], BF16, kind="ExternalInput").ap()
    pk = nc.dram_tensor("pk", [P, PK_N], BF16, kind="ExternalInput").ap()
    out = nc.dram_tensor("out", [P, 2], F32, kind="ExternalOutput").ap()

    add = mybir.AluOpType.add
    mult = mybir.AluOpType.mult
    bypass = mybir.AluOpType.bypass
    AX = mybir.AxisListType.X
    ACT = mybir.ActivationFunctionType

    with tile.TileContext(nc) as tc, ExitStack() as ctx:
        sp = ctx.enter_context(tc.tile_pool(name="small", bufs=1))
        xp = ctx.enter_context(tc.tile_pool(name="xp", bufs=NP))
        hp = ctx.enter_context(tc.tile_pool(name="hp", bufs=3))
        pp = ctx.enter_context(tc.tile_pool(name="psum", bufs=1, space="PSUM"))

        # constants share the stream's SWDGE queue (strict FIFO => no
        # starvation; the other queue gets single-digit B/ns while the
        # stream runs).  Order: w, x piece 0, everything else, x 1..N.
        pws = sp.tile([P, D], BF16)
        nc.gpsimd.dma_start(pws[:], pw)
        x_r = x.rearrange("b (c l) d -> (b c) (l d)", c=CH)
        xts = []
        xt0 = xp.tile([P, SZ[0] * D], BF16, tag="x")
        nc.gpsimd.dma_start(xt0[:], x_r[:, 0:SZ[0] * D])
        xts.append(xt0)
        pks = sp.tile([P, PK_N], BF16)
        nc.gpsimd.dma_start(pks[:], pk)
        for i in range(1, NP):
            xt = xp.tile([P, SZ[i] * D], BF16, tag="x")
            nc.gpsimd.dma_start(
                xt[:], x_r[:, OFF[i] * D:(OFF[i] + SZ[i]) * D]
            )
            xts.append(xt)

        umv = pks[:, PK_UM:PK_UM + P]
        mfv = pks[:, PK_MF:PK_MF + 2 * L].bitcast(F32)
        wmv = pks[:, PK_WM:PK_WM + 2 * L].bitcast(F32)
        amv = pks[:, PK_AM:PK_AM + 2 * L].bitcast(F32)

        # activation-table warm: no data deps, runs in the DMA shadow
        warm0 = sp.tile([P, 1], F32)
        nc.vector.memset(warm0[:], 1.0)
        warmo = sp.tile([P, 2], F32)
        nc.scalar.activation(warmo[:, 0:1], warm0[:], ACT.Exp)

        # replicate w MAXK times on-chip (bf16 copies run at 4x; w
        # lands well before the first x piece)
        wrep = sp.tile([P, MAXK * D], BF16)
        nc.vector.tensor_copy(wrep[:, 0:D], pws[:])
        rep = 1
        while rep < MAXK:
            n = min(rep, MAXK - rep)
            nc.vector.tensor_copy(
                wrep[:, rep * D:(rep + n) * D], wrep[:, 0:n * D]
            )
            rep += n
        w3 = wrep[:].rearrange("p (l d) -> p l d", d=D)

        # xw[p, t] = sum_d x[p, t, d] * w[d] — all on DVE: a gpsimd
        # tensor_tensor running concurrently halves DVE 2x throughput
        # (measured), so gpsimd gets no elementwise work at all
        xw = sp.tile([P, L], F32)
        res = sp.tile([P, 2], F32)
        for i in range(NP):
            k = SZ[i]
            x3 = xts[i][:].rearrange("p (l d) -> p l d", d=D)
            nc.vector.tensor_tensor(x3, x3, w3[:, 0:k, :], mult)
            h1 = hp.tile([P, MAXK * 128], BF16, tag="h1")
            h13 = h1[:, 0:k * 128].rearrange("p (l d) -> p l d", d=128)
            h2 = hp.tile([P, MAXK * 64], BF16, tag="h2")
            h23 = h2[:, 0:k * 64].rearrange("p (l d) -> p l d", d=64)
            nc.vector.tensor_tensor(h13, x3[:, :, 0:128], x3[:, :, 128:256], add)
            nc.vector.tensor_tensor(h23, h13[:, :, 0:64], h13[:, :, 64:128], add)
            nc.vector.tensor_reduce(
                xw[:, OFF[i]:OFF[i] + k], h23, axis=AX, op=add
            )
            if i == 3:
                # group count: cheap, inputs ready, DVE has slack here
                nc.vector.tensor_reduce(res[:, 1:2], mfv, axis=AX, op=add)

        # fold the mask in additively: valid cols unchanged, masked cols
        # pushed to ~-30 so exp gives ~1e-13 (suffix sums stay positive)
        nc.vector.tensor_tensor(xw[:], xw[:], amv, add)

        # masked exponentials; the accumulate IS the chunk total
        em = sp.tile([P, L], F32)
        tot = sp.tile([P, 1], F32)
        nc.scalar.activation(em[:], xw[:], ACT.Exp, accum_out=tot[:])

        # cross-chunk exclusive suffix of totals via one bf16 matmul
        tot_bf = sp.tile([P, 1], BF16)
        nc.vector.tensor_copy(tot_bf[:], tot[:])
        aps = pp.tile([P, 1], F32, tag="mm")
        nc.tensor.matmul(aps[:], umv, tot_bf[:], start=True, stop=True)

        # within-chunk suffix sums, seeded with the later-chunk total
        # (the scan reads its seed straight from PSUM)
        ss = sp.tile([P, L], F32)
        nc.vector.tensor_tensor_scan(
            ss[:][:, ::-1], em[:][:, ::-1], em[:][:, ::-1],
            initial=aps[:], op0=add, op1=bypass,
        )
        lt = sp.tile([P, L], F32)
        nc.scalar.activation(lt[:], ss[:], ACT.Ln)

        # loss terms: sum over valid groups of (ln(suffix) - xw); the
        # amask offset only lives where wm == 0, so it never contributes
        diff = sp.tile([P, L], F32)
        nc.vector.tensor_sub(diff[:], lt[:], xw[:])
        nc.vector.scalar_tensor_tensor(
            out=diff[:], in0=diff[:], scalar=1.0, in1=wmv,
            op0=bypass, op1=mult, accum_out=res[:, 0:1],
        )
        nc.sync.dma_start(out, res[:])

    nc.compile()
    return nc


def _host_consts():
    w_idx = np.arange(P)
    um = (
        (w_idx[:, None] // CH == w_idx[None, :] // CH)
        & (w_idx[:, None] % CH > w_idx[None, :] % CH)
    ).astype(np.float32)
    cm = np.ones((P, L), np.float32)
    cm[w_idx % CH == 0, 0] = 0.0
    return um, cm


def make_in_maps(enc, mask, w_fc):
    import ml_dtypes

    bf = ml_dtypes.bfloat16
    um, cm = _host_consts()
    w_bits = np.tile(w_fc[HID:].astype(bf).view(np.uint16)[None, :], (P, 1))
    um_bits = um.astype(bf).view(np.uint16)

    in_maps = []
    for c in range(NCORES):
        mf = mask[c * BS:(c + 1) * BS].reshape(P, L).astype(np.float32)
        wm = mf * cm
        am = (mf - 1.0) * NEGM
        pack = np.empty((P, PK_N), np.uint16)
        pack[:, PK_UM:PK_UM + P] = um_bits
        pack[:, PK_MF:PK_MF + 2 * L] = mf.view(np.uint16)
        pack[:, PK_WM:PK_WM + 2 * L] = wm.view(np.uint16)
        pack[:, PK_AM:PK_AM + 2 * L] = am.view(np.uint16)
        in_maps.append({
            "x": np.ascontiguousarray(enc[c * BS:(c + 1) * BS]),
            "pw": w_bits.view(bf),
            "pk": pack.view(bf),
        })
    return in_maps


def kernel(**inputs) -> np.ndarray:
    enc = np.ascontiguousarray(np.asarray(inputs["encoder_output"], np.float32))
    mask = np.ascontiguousarray(np.asarray(inputs["mask"], np.int32))
    w_fc = np.asarray(inputs["w_fc"], np.float32)

    if "nc" not in _cache:
        _cache["nc"] = _build_nc()
    nc = _cache["nc"]

    res = bass_utils.run_bass_kernel_spmd(
        nc, make_in_maps(enc, mask, w_fc), core_ids=list(range(NCORES))
    )
    o = np.stack([r["out"] for r in res.results]).astype(np.float64)
    num = o[:, :, 0].sum()
    den = o[:, :, 1].sum() - B
    return np.asarray(num / den, dtype=np.float32)


# revision 22
# speedup vs baseline: 1.0125x; 1.0125x over previous
"""Trainium2 Bass kernel for nn_DLI_loss_full.

Key algebraic fact: logits[b,j,k] = hw[b,j] + xw[b,k] and the loss is
sum(lse - tgt) over valid groups, so the hw[b,j] term (the whole LSTM
path) cancels exactly:

    per_group[b,j] = log(sum_{k=j+1}^{len_b-1} exp(xw[b,k])) - xw[b,j+1]
    loss = sum(per_group) / sum_b(len_b - 1)

with xw = encoder_output @ w_fc[HID:].  The kernel only streams
encoder_output once (memory-bound; ~6.3MB/core, ~311 B/ns sustained).

Structure (trace-driven, v3):
  * stream DMAs cast f32->bf16 in the SDMA datapath (SWDGE path; HBM
    reads unchanged, SBUF writes halved) so the multiply runs on DVE in
    bf16 2x perf mode and the 256->1 reduction runs as two bf16 2x
    tree-add halvings plus a 64-wide tensor_reduce.
  * ALL constants (w replicated, the chunk-combine matrix, and the
    host-precomputed mask tensors mf/wm/amask) ride in ONE packed
    tensor issued FIFO-first on the SAME SWDGE queue as the stream: a
    queue that isn't the majority-traffic queue gets starved to
    single-digit B/ns while the stream runs (measured), so sharing the
    stream's queue is the only ordering guarantee.
  * piece sizes are non-uniform (4,10,10,10,10,4 timesteps of the 48
    per chunk): a small first piece starts the DVE pipeline early, a
    small last piece shrinks the after-last-byte compute tail, and few
    big middle pieces amortize the ~0.5us/piece DVE instruction
    overhead.  gpsimd gets NO elementwise work: a gpsimd tensor_tensor
    running concurrently halves DVE 2x-mode throughput (measured).
  * the mask is folded in additively ((mf-1)*30) before a single
    exp-with-accumulate on the scalar engine: the accum IS the chunk
    total, and masked exps are e^-30 (not 0) so every suffix ln stays
    finite with no epsilon pass.
  * a dummy Ln that READS em[0:1] is placed right after the exp: the
    data dependency pins it there (an input-free warm gets hoisted by
    the scheduler into the exp table's residency window, causing a
    table-load ping-pong, also measured), so the Ln table load overlaps
    the matmul/scan chain instead of the critical path.
"""

from contextlib import ExitStack

import numpy as np

import concourse.bacc as bacc
import concourse.mybir as mybir
import concourse.tile as tile
from concourse import bass_utils

B, T, D, HID = 128, 384, 256, 256
NCORES = 8
BS = B // NCORES            # 16 batches per core
CH = 8                      # chunks per sequence
L = T // CH                 # 48 timesteps per chunk
P = BS * CH                 # 128 partitions
SZ = (4, 10, 10, 10, 10, 4)        # timesteps per piece (sum = L)
OFF = tuple(np.cumsum((0,) + SZ)[:len(SZ)])
NP = len(SZ)
MAXK = max(SZ)
F32 = mybir.dt.float32
BF16 = mybir.dt.bfloat16
NEGM = 30.0                 # additive mask depth: exp(xw-30) ~ 1e-13

# w rides alone ahead of the first x piece (it gates the whole DVE
# pipeline); everything else (only needed from mid-kernel on) follows
# the first x piece.  In bf16 columns:
PK_UM = 0                   # chunk-combine matrix         [P, P]      bf16
PK_MF = PK_UM + P           # mask as f32                  [P, L]      f32
PK_WM = PK_MF + 2 * L       # group-validity weights f32   [P, L]      f32
PK_AM = PK_WM + 2 * L       # additive mask (mf-1)*30 f32  [P, L]      f32
PK_N = PK_AM + 2 * L

_cache = {}


def _joint_act_tables(arch, _orig=bacc.get_activation_tables):
    """Steer the act-table-load pass to the single set that holds BOTH
    exp and ln (natural_log_exp_and_others): the per-function greedy
    choice otherwise loads one set per function and the second ~1.3us
    table load lands on the critical path between the exp and the ln.
    Set names/order (and therefore act_func_set_ids) are preserved; the
    other sets are just emptied so they can never be chosen.  Falls
    back to the untouched tables if no joint set exists."""
    d = _orig(arch)
    exp = mybir.ActivationFunctionType.Exp
    ln = mybir.ActivationFunctionType.Ln
    joint = [n for n, fns in d.items() if exp in fns and ln in fns]
    if joint:
        keep = joint[0]
        for n in d:
            if n != keep:
                d[n] = set()
    return d


bacc.get_activation_tables = _joint_act_tables


def _build_nc():
    nc = bacc.Bacc(
        "TRN2", target_bir_lowering=False, debug=False, num_devices=NCORES
    )
    x = nc.dram_tensor("x", [BS, T, D], F32, kind="ExternalInput").ap()
    pw = nc.dram_tensor("pw", [P, D], BF16, kind="ExternalInput").ap()
    pk = nc.dram_tensor("pk", [P, PK_N], BF16, kind="ExternalInput").ap()
    out = nc.dram_tensor("out", [P, 2], F32, kind="ExternalOutput").ap()

    add = mybir.AluOpType.add
    mult = mybir.AluOpType.mult
    bypass = mybir.AluOpType.bypass
    AX = mybir.AxisListType.X
    ACT = mybir.ActivationFunctionType

    with tile.TileContext(nc) as tc, ExitStack() as ctx:
        sp = ctx.enter_context(tc.tile_pool(name="small", bufs=1))
        xp = ctx.enter_context(tc.tile_pool(name="xp", bufs=NP))
        hp = ctx.enter_context(tc.tile_pool(name="hp", bufs=3))
        pp = ctx.enter_context(tc.tile_pool(name="psum", bufs=1, space="PSUM"))

        # constants share the stream's SWDGE queue (strict FIFO => no
        # starvation; the other queue gets single-digit B/ns while the
        # stream runs).  Order: w, x piece 0, everything else, x 1..N.
        pws = sp.tile([P, D], BF16)
        nc.gpsimd.dma_start(pws[:], pw)
        x_r = x.rearrange("b (c l) d -> (b c) (l d)", c=CH)
        xts = []
        xt0 = xp.tile([P, SZ[0] * D], BF16, tag="x")
        nc.gpsimd.dma_start(xt0[:], x_r[:, 0:SZ[0] * D])
        xts.append(xt0)
        pks = sp.tile([P, PK_N], BF16)
        nc.gpsimd.dma_start(pks[:], pk)
        for i in range(1, NP):
            xt = xp.tile([P, SZ[i] * D], BF16, tag="x")
            nc.gpsimd.dma_start(
                xt[:], x_r[:, OFF[i] * D:(OFF[i] + SZ[i]) * D]
            )
            xts.append(xt)

        umv = pks[:, PK_UM:PK_UM + P]
        mfv = pks[:, PK_MF:PK_MF + 2 * L].bitcast(F32)
        wmv = pks[:, PK_WM:PK_WM + 2 * L].bitcast(F32)
        amv = pks[:, PK_AM:PK_AM + 2 * L].bitcast(F32)

        # activation-table warm: no data deps, runs in the DMA shadow
        warm0 = sp.tile([P, 1], F32)
        nc.vector.memset(warm0[:], 1.0)
        warmo = sp.tile([P, 2], F32)
        nc.scalar.activation(warmo[:, 0:1], warm0[:], ACT.Exp)

        # replicate w MAXK times on-chip (bf16 copies run at 4x; w
        # lands well before the first x piece)
        wrep = sp.tile([P, MAXK * D], BF16)
        nc.vector.tensor_copy(wrep[:, 0:D], pws[:])
        rep = 1
        while rep < MAXK:
            n = min(rep, MAXK - rep)
            nc.vector.tensor_copy(
                wrep[:, rep * D:(rep + n) * D], wrep[:, 0:n * D]
            )
            rep += n
        w3 = wrep[:].rearrange("p (l d) -> p l d", d=D)

        # xw[p, t] = sum_d x[p, t, d] * w[d] — all on DVE: a gpsimd
        # tensor_tensor running concurrently halves DVE 2x throughput
        # (measured), so gpsimd gets no elementwise work at all
        xw = sp.tile([P, L], F32)
        res = sp.tile([P, 2], F32)
        for i in range(NP):
            k = SZ[i]
            x3 = xts[i][:].rearrange("p (l d) -> p l d", d=D)
            nc.vector.tensor_tensor(x3, x3, w3[:, 0:k, :], mult)
            h1 = hp.tile([P, MAXK * 128], BF16, tag="h1")
            h13 = h1[:, 0:k * 128].rearrange("p (l d) -> p l d", d=128)
            h2 = hp.tile([P, MAXK * 64], BF16, tag="h2")
            h23 = h2[:, 0:k * 64].rearrange("p (l d) -> p l d", d=64)
            nc.vector.tensor_tensor(h13, x3[:, :, 0:128], x3[:, :, 128:256], add)
            nc.vector.tensor_tensor(h23, h13[:, :, 0:64], h13[:, :, 64:128], add)
            nc.vector.tensor_reduce(
                xw[:, OFF[i]:OFF[i] + k], h23, axis=AX, op=add
            )
            if i == 3:
                # group count: cheap, inputs ready, DVE has slack here
                nc.vector.tensor_reduce(res[:, 1:2], mfv, axis=AX, op=add)

        # fold the mask in additively: valid cols unchanged, masked cols
        # pushed to ~-30 so exp gives ~1e-13 (suffix sums stay positive)
        nc.vector.tensor_tensor(xw[:], xw[:], amv, add)

        # masked exponentials; the accumulate IS the chunk total
        em = sp.tile([P, L], F32)
        tot = sp.tile([P, 1], F32)
        nc.scalar.activation(em[:], xw[:], ACT.Exp, accum_out=tot[:])

        # cross-chunk exclusive suffix of totals via one bf16 matmul
        tot_bf = sp.tile([P, 1], BF16)
        nc.vector.tensor_copy(tot_bf[:], tot[:])
        aps = pp.tile([P, 1], F32, tag="mm")
        nc.tensor.matmul(aps[:], umv, tot_bf[:], start=True, stop=True)

        # within-chunk suffix sums, seeded with the later-chunk total
        # (the scan reads its seed straight from PSUM)
        ss = sp.tile([P, L], F32)
        nc.vector.tensor_tensor_scan(
            ss[:][:, ::-1], em[:][:, ::-1], em[:][:, ::-1],
            initial=aps[:], op0=add, op1=bypass,
        )
        lt = sp.tile([P, L], F32)
        nc.scalar.activation(lt[:], ss[:], ACT.Ln)

        # loss terms: sum over valid groups of (ln(suffix) - xw); the
        # amask offset only lives where wm == 0, so it never contributes
        diff = sp.tile([P, L], F32)
        nc.vector.tensor_sub(diff[:], lt[:], xw[:])
        nc.vector.scalar_tensor_tensor(
            out=diff[:], in0=diff[:], scalar=1.0, in1=wmv,
            op0=bypass, op1=mult, accum_out=res[:, 0:1],
        )
        nc.sync.dma_start(out, res[:], single_packet=True)

    nc.compile()
    return nc


def _host_consts():
    w_idx = np.arange(P)
    um = (
        (w_idx[:, None] // CH == w_idx[None, :] // CH)
        & (w_idx[:, None] % CH > w_idx[None, :] % CH)
    ).astype(np.float32)
    cm = np.ones((P, L), np.float32)
    cm[w_idx % CH == 0, 0] = 0.0
    return um, cm


def make_in_maps(enc, mask, w_fc):
    import ml_dtypes

    bf = ml_dtypes.bfloat16
    um, cm = _host_consts()
    w_bits = np.tile(w_fc[HID:].astype(bf).view(np.uint16)[None, :], (P, 1))
    um_bits = um.astype(bf).view(np.uint16)

    in_maps = []
    for c in range(NCORES):
        mf = mask[c * BS:(c + 1) * BS].reshape(P, L).astype(np.float32)
        wm = mf * cm
        am = (mf - 1.0) * NEGM
        pack = np.empty((P, PK_N), np.uint16)
        pack[:, PK_UM:PK_UM + P] = um_bits
        pack[:, PK_MF:PK_MF + 2 * L] = mf.view(np.uint16)
        pack[:, PK_WM:PK_WM + 2 * L] = wm.view(np.uint16)
        pack[:, PK_AM:PK_AM + 2 * L] = am.view(np.uint16)
        in_maps.append({
            "x": np.ascontiguousarray(enc[c * BS:(c + 1) * BS]),
            "pw": w_bits.view(bf),
            "pk": pack.view(bf),
        })
    return in_maps


def kernel(**inputs) -> np.ndarray:
    enc = np.ascontiguousarray(np.asarray(inputs["encoder_output"], np.float32))
    mask = np.ascontiguousarray(np.asarray(inputs["mask"], np.int32))
    w_fc = np.asarray(inputs["w_fc"], np.float32)

    if "nc" not in _cache:
        _cache["nc"] = _build_nc()
    nc = _cache["nc"]

    res = bass_utils.run_bass_kernel_spmd(
        nc, make_in_maps(enc, mask, w_fc), core_ids=list(range(NCORES))
    )
    o = np.stack([r["out"] for r in res.results]).astype(np.float64)
    num = o[:, :, 0].sum()
    den = o[:, :, 1].sum() - B
    return np.asarray(num / den, dtype=np.float32)


# revision 25
# speedup vs baseline: 1.0222x; 1.0095x over previous
"""Trainium2 Bass kernel for nn_DLI_loss_full.

Key algebraic fact: logits[b,j,k] = hw[b,j] + xw[b,k] and the loss is
sum(lse - tgt) over valid groups, so the hw[b,j] term (the whole LSTM
path) cancels exactly:

    per_group[b,j] = log(sum_{k=j+1}^{len_b-1} exp(xw[b,k])) - xw[b,j+1]
    loss = sum(per_group) / sum_b(len_b - 1)

with xw = encoder_output @ w_fc[HID:].  The kernel only streams
encoder_output once (memory-bound; ~6.3MB/core, ~311 B/ns sustained).

Structure (trace-driven, v3):
  * stream DMAs cast f32->bf16 in the SDMA datapath (SWDGE path; HBM
    reads unchanged, SBUF writes halved) so the multiply runs on DVE in
    bf16 2x perf mode and the 256->1 reduction runs as two bf16 2x
    tree-add halvings plus a 64-wide tensor_reduce.
  * ALL constants (w replicated, the chunk-combine matrix, and the
    host-precomputed mask tensors mf/wm/amask) ride in ONE packed
    tensor issued FIFO-first on the SAME SWDGE queue as the stream: a
    queue that isn't the majority-traffic queue gets starved to
    single-digit B/ns while the stream runs (measured), so sharing the
    stream's queue is the only ordering guarantee.
  * piece sizes are non-uniform (4,10,10,10,10,4 timesteps of the 48
    per chunk): a small first piece starts the DVE pipeline early, a
    small last piece shrinks the after-last-byte compute tail, and few
    big middle pieces amortize the ~0.5us/piece DVE instruction
    overhead.  gpsimd gets NO elementwise work: a gpsimd tensor_tensor
    running concurrently halves DVE 2x-mode throughput (measured).
  * the mask is folded in additively ((mf-1)*30) before a single
    exp-with-accumulate on the scalar engine: the accum IS the chunk
    total, and masked exps are e^-30 (not 0) so every suffix ln stays
    finite with no epsilon pass.
  * a dummy Ln that READS em[0:1] is placed right after the exp: the
    data dependency pins it there (an input-free warm gets hoisted by
    the scheduler into the exp table's residency window, causing a
    table-load ping-pong, also measured), so the Ln table load overlaps
    the matmul/scan chain instead of the critical path.
"""

from contextlib import ExitStack

import numpy as np

import concourse.bacc as bacc
import concourse.mybir as mybir
import concourse.tile as tile
from concourse import bass_utils

B, T, D, HID = 128, 384, 256, 256
NCORES = 8
BS = B // NCORES            # 16 batches per core
CH = 8                      # chunks per sequence
L = T // CH                 # 48 timesteps per chunk
P = BS * CH                 # 128 partitions
SZ = (4, 10, 10, 10, 10, 3, 1)     # timesteps per piece (sum = L)
OFF = tuple(np.cumsum((0,) + SZ)[:len(SZ)])
NP = len(SZ)
MAXK = max(SZ)
F32 = mybir.dt.float32
BF16 = mybir.dt.bfloat16
NEGM = 30.0                 # additive mask depth: exp(xw-30) ~ 1e-13

# w rides alone ahead of the first x piece (it gates the whole DVE
# pipeline); everything else (only needed from mid-kernel on) follows
# the first x piece.  In bf16 columns:
PK_UM = 0                   # chunk-combine matrix         [P, P]      bf16
PK_MF = PK_UM + P           # mask as f32                  [P, L]      f32
PK_WM = PK_MF + 2 * L       # group-validity weights f32   [P, L]      f32
PK_AM = PK_WM + 2 * L       # additive mask (mf-1)*30 f32  [P, L]      f32
PK_N = PK_AM + 2 * L

_cache = {}


def _joint_act_tables(arch, _orig=bacc.get_activation_tables):
    """Steer the act-table-load pass to the single set that holds BOTH
    exp and ln (natural_log_exp_and_others): the per-function greedy
    choice otherwise loads one set per function and the second ~1.3us
    table load lands on the critical path between the exp and the ln.
    Set names/order (and therefore act_func_set_ids) are preserved; the
    other sets are just emptied so they can never be chosen.  Falls
    back to the untouched tables if no joint set exists."""
    d = _orig(arch)
    exp = mybir.ActivationFunctionType.Exp
    ln = mybir.ActivationFunctionType.Ln
    joint = [n for n, fns in d.items() if exp in fns and ln in fns]
    if joint:
        keep = joint[0]
        for n in d:
            if n != keep:
                d[n] = set()
    return d


bacc.get_activation_tables = _joint_act_tables


def _build_nc():
    nc = bacc.Bacc(
        "TRN2", target_bir_lowering=False, debug=False, num_devices=NCORES
    )
    x = nc.dram_tensor("x", [BS, T, D], F32, kind="ExternalInput").ap()
    pw = nc.dram_tensor("pw", [P, D], BF16, kind="ExternalInput").ap()
    pk = nc.dram_tensor("pk", [P, PK_N], BF16, kind="ExternalInput").ap()
    out = nc.dram_tensor("out", [P, 2], F32, kind="ExternalOutput").ap()

    add = mybir.AluOpType.add
    mult = mybir.AluOpType.mult
    bypass = mybir.AluOpType.bypass
    AX = mybir.AxisListType.X
    ACT = mybir.ActivationFunctionType

    with tile.TileContext(nc) as tc, ExitStack() as ctx:
        sp = ctx.enter_context(tc.tile_pool(name="small", bufs=1))
        xp = ctx.enter_context(tc.tile_pool(name="xp", bufs=NP))
        hp = ctx.enter_context(tc.tile_pool(name="hp", bufs=3))
        pp = ctx.enter_context(tc.tile_pool(name="psum", bufs=1, space="PSUM"))

        # w rides the scalar HWDGE queue: issued first, its 64KB finish
        # before the SWDGE stream floods the SDMA engines (a minority
        # queue is starved to single-digit B/ns only once the stream is
        # running).  The mask pack shares the stream's SWDGE queue right
        # after piece 0 (strict FIFO => guaranteed arrival).
        pws = sp.tile([P, D], BF16)
        nc.scalar.dma_start(pws[:], pw)
        x_r = x.rearrange("b (c l) d -> (b c) (l d)", c=CH)
        xts = []
        xt0 = xp.tile([P, SZ[0] * D], BF16, tag="x")
        nc.gpsimd.dma_start(xt0[:], x_r[:, 0:SZ[0] * D])
        xts.append(xt0)
        pks = sp.tile([P, PK_N], BF16)
        nc.gpsimd.dma_start(pks[:], pk)
        for i in range(1, NP):
            xt = xp.tile([P, SZ[i] * D], BF16, tag="x")
            nc.gpsimd.dma_start(
                xt[:], x_r[:, OFF[i] * D:(OFF[i] + SZ[i]) * D]
            )
            xts.append(xt)

        umv = pks[:, PK_UM:PK_UM + P]
        mfv = pks[:, PK_MF:PK_MF + 2 * L].bitcast(F32)
        wmv = pks[:, PK_WM:PK_WM + 2 * L].bitcast(F32)
        amv = pks[:, PK_AM:PK_AM + 2 * L].bitcast(F32)

        # activation-table warm: no data deps, runs in the DMA shadow
        warm0 = sp.tile([P, 1], F32)
        nc.vector.memset(warm0[:], 1.0)
        warmo = sp.tile([P, 2], F32)
        nc.scalar.activation(warmo[:, 0:1], warm0[:], ACT.Exp)

        # replicate w MAXK times on-chip (bf16 copies run at 4x; w
        # lands well before the first x piece)
        wrep = sp.tile([P, MAXK * D], BF16)
        nc.vector.tensor_copy(wrep[:, 0:D], pws[:])
        rep = 1
        while rep < MAXK:
            n = min(rep, MAXK - rep)
            nc.vector.tensor_copy(
                wrep[:, rep * D:(rep + n) * D], wrep[:, 0:n * D]
            )
            rep += n
        w3 = wrep[:].rearrange("p (l d) -> p l d", d=D)

        # xw[p, t] = sum_d x[p, t, d] * w[d] — all on DVE: a gpsimd
        # tensor_tensor running concurrently halves DVE 2x throughput
        # (measured), so gpsimd gets no elementwise work at all
        xw = sp.tile([P, L], F32)
        res = sp.tile([P, 2], F32)
        for i in range(NP):
            k = SZ[i]
            x3 = xts[i][:].rearrange("p (l d) -> p l d", d=D)
            nc.vector.tensor_tensor(x3, x3, w3[:, 0:k, :], mult)
            if k <= 2:
                # tiny tail piece: the tree's per-instruction overhead
                # exceeds its savings, reduce directly
                nc.vector.tensor_reduce(
                    xw[:, OFF[i]:OFF[i] + k], x3, axis=AX, op=add
                )
            else:
                h1 = hp.tile([P, MAXK * 128], BF16, tag="h1")
                h13 = h1[:, 0:k * 128].rearrange("p (l d) -> p l d", d=128)
                h2 = hp.tile([P, MAXK * 64], BF16, tag="h2")
                h23 = h2[:, 0:k * 64].rearrange("p (l d) -> p l d", d=64)
                nc.vector.tensor_tensor(h13, x3[:, :, 0:128], x3[:, :, 128:256], add)
                nc.vector.tensor_tensor(h23, h13[:, :, 0:64], h13[:, :, 64:128], add)
                nc.vector.tensor_reduce(
                    xw[:, OFF[i]:OFF[i] + k], h23, axis=AX, op=add
                )
            if i == 3:
                # group count: cheap, inputs ready, DVE has slack here
                nc.vector.tensor_reduce(res[:, 1:2], mfv, axis=AX, op=add)

        # fold the mask in additively: valid cols unchanged, masked cols
        # pushed to ~-30 so exp gives ~1e-13 (suffix sums stay positive)
        nc.vector.tensor_tensor(xw[:], xw[:], amv, add)

        # masked exponentials; the accumulate IS the chunk total
        em = sp.tile([P, L], F32)
        tot = sp.tile([P, 1], F32)
        nc.scalar.activation(em[:], xw[:], ACT.Exp, accum_out=tot[:])

        # cross-chunk exclusive suffix of totals via one bf16 matmul
        tot_bf = sp.tile([P, 1], BF16)
        nc.vector.tensor_copy(tot_bf[:], tot[:])
        aps = pp.tile([P, 1], F32, tag="mm")
        nc.tensor.matmul(aps[:], umv, tot_bf[:], start=True, stop=True)

        # within-chunk suffix sums, seeded with the later-chunk total
        # (the scan reads its seed straight from PSUM)
        ss = sp.tile([P, L], F32)
        nc.vector.tensor_tensor_scan(
            ss[:][:, ::-1], em[:][:, ::-1], em[:][:, ::-1],
            initial=aps[:], op0=add, op1=bypass,
        )
        lt = sp.tile([P, L], F32)
        nc.scalar.activation(lt[:], ss[:], ACT.Ln)

        # loss terms: sum over valid groups of (ln(suffix) - xw); the
        # amask offset only lives where wm == 0, so it never contributes
        diff = sp.tile([P, L], F32)
        nc.vector.tensor_sub(diff[:], lt[:], xw[:])
        nc.vector.scalar_tensor_tensor(
            out=diff[:], in0=diff[:], scalar=1.0, in1=wmv,
            op0=bypass, op1=mult, accum_out=res[:, 0:1],
        )
        nc.sync.dma_start(out, res[:], single_packet=True)

    nc.compile()
    return nc


def _host_consts():
    w_idx = np.arange(P)
    um = (
        (w_idx[:, None] // CH == w_idx[None, :] // CH)
        & (w_idx[:, None] % CH > w_idx[None, :] % CH)
    ).astype(np.float32)
    cm = np.ones((P, L), np.float32)
    cm[w_idx % CH == 0, 0] = 0.0
    return um, cm


def make_in_maps(enc, mask, w_fc):
    import ml_dtypes

    bf = ml_dtypes.bfloat16
    um, cm = _host_consts()
    w_bits = np.tile(w_fc[HID:].astype(bf).view(np.uint16)[None, :], (P, 1))
    um_bits = um.astype(bf).view(np.uint16)

    in_maps = []
    for c in range(NCORES):
        mf = mask[c * BS:(c + 1) * BS].reshape(P, L).astype(np.float32)
        wm = mf * cm
        am = (mf - 1.0) * NEGM
        pack = np.empty((P, PK_N), np.uint16)
        pack[:, PK_UM:PK_UM + P] = um_bits
        pack[:, PK_MF:PK_MF + 2 * L] = mf.view(np.uint16)
        pack[:, PK_WM:PK_WM + 2 * L] = wm.view(np.uint16)
        pack[:, PK_AM:PK_AM + 2 * L] = am.view(np.uint16)
        in_maps.append({
            "x": np.ascontiguousarray(enc[c * BS:(c + 1) * BS]),
            "pw": w_bits.view(bf),
            "pk": pack.view(bf),
        })
    return in_maps


def kernel(**inputs) -> np.ndarray:
    enc = np.ascontiguousarray(np.asarray(inputs["encoder_output"], np.float32))
    mask = np.ascontiguousarray(np.asarray(inputs["mask"], np.int32))
    w_fc = np.asarray(inputs["w_fc"], np.float32)

    if "nc" not in _cache:
        _cache["nc"] = _build_nc()
    nc = _cache["nc"]

    res = bass_utils.run_bass_kernel_spmd(
        nc, make_in_maps(enc, mask, w_fc), core_ids=list(range(NCORES))
    )
    o = np.stack([r["out"] for r in res.results]).astype(np.float64)
    num = o[:, :, 0].sum()
    den = o[:, :, 1].sum() - B
    return np.asarray(num / den, dtype=np.float32)


# revision 26
# speedup vs baseline: 1.0242x; 1.0020x over previous
"""Trainium2 Bass kernel for nn_DLI_loss_full.

Key algebraic fact: logits[b,j,k] = hw[b,j] + xw[b,k] and the loss is
sum(lse - tgt) over valid groups, so the hw[b,j] term (the whole LSTM
path) cancels exactly:

    per_group[b,j] = log(sum_{k=j+1}^{len_b-1} exp(xw[b,k])) - xw[b,j+1]
    loss = sum(per_group) / sum_b(len_b - 1)

with xw = encoder_output @ w_fc[HID:].  The kernel only streams
encoder_output once (memory-bound; ~6.3MB/core, ~311 B/ns sustained).

Structure (trace-driven, v3):
  * stream DMAs cast f32->bf16 in the SDMA datapath (SWDGE path; HBM
    reads unchanged, SBUF writes halved) so the multiply runs on DVE in
    bf16 2x perf mode and the 256->1 reduction runs as two bf16 2x
    tree-add halvings plus a 64-wide tensor_reduce.
  * ALL constants (w replicated, the chunk-combine matrix, and the
    host-precomputed mask tensors mf/wm/amask) ride in ONE packed
    tensor issued FIFO-first on the SAME SWDGE queue as the stream: a
    queue that isn't the majority-traffic queue gets starved to
    single-digit B/ns while the stream runs (measured), so sharing the
    stream's queue is the only ordering guarantee.
  * piece sizes are non-uniform (4,10,10,10,10,4 timesteps of the 48
    per chunk): a small first piece starts the DVE pipeline early, a
    small last piece shrinks the after-last-byte compute tail, and few
    big middle pieces amortize the ~0.5us/piece DVE instruction
    overhead.  gpsimd gets NO elementwise work: a gpsimd tensor_tensor
    running concurrently halves DVE 2x-mode throughput (measured).
  * the mask is folded in additively ((mf-1)*30) before a single
    exp-with-accumulate on the scalar engine: the accum IS the chunk
    total, and masked exps are e^-30 (not 0) so every suffix ln stays
    finite with no epsilon pass.
  * a dummy Ln that READS em[0:1] is placed right after the exp: the
    data dependency pins it there (an input-free warm gets hoisted by
    the scheduler into the exp table's residency window, causing a
    table-load ping-pong, also measured), so the Ln table load overlaps
    the matmul/scan chain instead of the critical path.
"""

from contextlib import ExitStack

import numpy as np

import concourse.bacc as bacc
import concourse.mybir as mybir
import concourse.tile as tile
from concourse import bass_utils

B, T, D, HID = 128, 384, 256, 256
NCORES = 8
BS = B // NCORES            # 16 batches per core
CH = 8                      # chunks per sequence
L = T // CH                 # 48 timesteps per chunk
P = BS * CH                 # 128 partitions
SZ = (4, 10, 10, 10, 11, 2, 1)     # timesteps per piece (sum = L)
OFF = tuple(np.cumsum((0,) + SZ)[:len(SZ)])
NP = len(SZ)
MAXK = max(SZ)
F32 = mybir.dt.float32
BF16 = mybir.dt.bfloat16
NEGM = 30.0                 # additive mask depth: exp(xw-30) ~ 1e-13

# w rides alone ahead of the first x piece (it gates the whole DVE
# pipeline); everything else (only needed from mid-kernel on) follows
# the first x piece.  In bf16 columns:
PK_UM = 0                   # chunk-combine matrix         [P, P]      bf16
PK_MF = PK_UM + P           # mask as f32                  [P, L]      f32
PK_WM = PK_MF + 2 * L       # group-validity weights f32   [P, L]      f32
PK_AM = PK_WM + 2 * L       # additive mask (mf-1)*30 f32  [P, L]      f32
PK_N = PK_AM + 2 * L

_cache = {}


def _joint_act_tables(arch, _orig=bacc.get_activation_tables):
    """Steer the act-table-load pass to the single set that holds BOTH
    exp and ln (natural_log_exp_and_others): the per-function greedy
    choice otherwise loads one set per function and the second ~1.3us
    table load lands on the critical path between the exp and the ln.
    Set names/order (and therefore act_func_set_ids) are preserved; the
    other sets are just emptied so they can never be chosen.  Falls
    back to the untouched tables if no joint set exists."""
    d = _orig(arch)
    exp = mybir.ActivationFunctionType.Exp
    ln = mybir.ActivationFunctionType.Ln
    joint = [n for n, fns in d.items() if exp in fns and ln in fns]
    if joint:
        keep = joint[0]
        for n in d:
            if n != keep:
                d[n] = set()
    return d


bacc.get_activation_tables = _joint_act_tables


def _build_nc():
    nc = bacc.Bacc(
        "TRN2", target_bir_lowering=False, debug=False, num_devices=NCORES
    )
    x = nc.dram_tensor("x", [BS, T, D], F32, kind="ExternalInput").ap()
    pw = nc.dram_tensor("pw", [P, D], BF16, kind="ExternalInput").ap()
    pk = nc.dram_tensor("pk", [P, PK_N], BF16, kind="ExternalInput").ap()
    out = nc.dram_tensor("out", [P, 2], F32, kind="ExternalOutput").ap()

    add = mybir.AluOpType.add
    mult = mybir.AluOpType.mult
    bypass = mybir.AluOpType.bypass
    AX = mybir.AxisListType.X
    ACT = mybir.ActivationFunctionType

    with tile.TileContext(nc) as tc, ExitStack() as ctx:
        sp = ctx.enter_context(tc.tile_pool(name="small", bufs=1))
        xp = ctx.enter_context(tc.tile_pool(name="xp", bufs=NP))
        hp = ctx.enter_context(tc.tile_pool(name="hp", bufs=3))
        pp = ctx.enter_context(tc.tile_pool(name="psum", bufs=1, space="PSUM"))

        # w rides the scalar HWDGE queue: issued first, its 64KB finish
        # before the SWDGE stream floods the SDMA engines (a minority
        # queue is starved to single-digit B/ns only once the stream is
        # running).  The mask pack shares the stream's SWDGE queue right
        # after piece 0 (strict FIFO => guaranteed arrival).
        pws = sp.tile([P, D], BF16)
        nc.scalar.dma_start(pws[:], pw)
        x_r = x.rearrange("b (c l) d -> (b c) (l d)", c=CH)
        xts = []
        xt0 = xp.tile([P, SZ[0] * D], BF16, tag="x")
        nc.gpsimd.dma_start(xt0[:], x_r[:, 0:SZ[0] * D])
        xts.append(xt0)
        pks = sp.tile([P, PK_N], BF16)
        nc.gpsimd.dma_start(pks[:], pk)
        for i in range(1, NP):
            xt = xp.tile([P, SZ[i] * D], BF16, tag="x")
            nc.gpsimd.dma_start(
                xt[:], x_r[:, OFF[i] * D:(OFF[i] + SZ[i]) * D]
            )
            xts.append(xt)

        umv = pks[:, PK_UM:PK_UM + P]
        mfv = pks[:, PK_MF:PK_MF + 2 * L].bitcast(F32)
        wmv = pks[:, PK_WM:PK_WM + 2 * L].bitcast(F32)
        amv = pks[:, PK_AM:PK_AM + 2 * L].bitcast(F32)

        # activation-table warm: no data deps, runs in the DMA shadow
        warm0 = sp.tile([P, 1], F32)
        nc.vector.memset(warm0[:], 1.0)
        warmo = sp.tile([P, 2], F32)
        nc.scalar.activation(warmo[:, 0:1], warm0[:], ACT.Exp)

        # replicate w MAXK times on-chip (bf16 copies run at 4x; w
        # lands well before the first x piece)
        wrep = sp.tile([P, MAXK * D], BF16)
        nc.vector.tensor_copy(wrep[:, 0:D], pws[:])
        rep = 1
        while rep < MAXK:
            n = min(rep, MAXK - rep)
            nc.vector.tensor_copy(
                wrep[:, rep * D:(rep + n) * D], wrep[:, 0:n * D]
            )
            rep += n
        w3 = wrep[:].rearrange("p (l d) -> p l d", d=D)

        # xw[p, t] = sum_d x[p, t, d] * w[d] — all on DVE: a gpsimd
        # tensor_tensor running concurrently halves DVE 2x throughput
        # (measured), so gpsimd gets no elementwise work at all
        xw = sp.tile([P, L], F32)
        res = sp.tile([P, 2], F32)
        for i in range(NP):
            k = SZ[i]
            x3 = xts[i][:].rearrange("p (l d) -> p l d", d=D)
            nc.vector.tensor_tensor(x3, x3, w3[:, 0:k, :], mult)
            if k <= 2:
                # tiny tail piece: the tree's per-instruction overhead
                # exceeds its savings, reduce directly
                nc.vector.tensor_reduce(
                    xw[:, OFF[i]:OFF[i] + k], x3, axis=AX, op=add
                )
            else:
                h1 = hp.tile([P, MAXK * 128], BF16, tag="h1")
                h13 = h1[:, 0:k * 128].rearrange("p (l d) -> p l d", d=128)
                h2 = hp.tile([P, MAXK * 64], BF16, tag="h2")
                h23 = h2[:, 0:k * 64].rearrange("p (l d) -> p l d", d=64)
                nc.vector.tensor_tensor(h13, x3[:, :, 0:128], x3[:, :, 128:256], add)
                nc.vector.tensor_tensor(h23, h13[:, :, 0:64], h13[:, :, 64:128], add)
                nc.vector.tensor_reduce(
                    xw[:, OFF[i]:OFF[i] + k], h23, axis=AX, op=add
                )
            if i == 3:
                # group count: cheap, inputs ready, DVE has slack here
                nc.vector.tensor_reduce(res[:, 1:2], mfv, axis=AX, op=add)

        # fold the mask in additively: valid cols unchanged, masked cols
        # pushed to ~-30 so exp gives ~1e-13 (suffix sums stay positive)
        nc.vector.tensor_tensor(xw[:], xw[:], amv, add)

        # masked exponentials; the accumulate IS the chunk total
        em = sp.tile([P, L], F32)
        tot = sp.tile([P, 1], F32)
        nc.scalar.activation(em[:], xw[:], ACT.Exp, accum_out=tot[:])

        # cross-chunk exclusive suffix of totals via one bf16 matmul
        tot_bf = sp.tile([P, 1], BF16)
        nc.vector.tensor_copy(tot_bf[:], tot[:])
        aps = pp.tile([P, 1], F32, tag="mm")
        nc.tensor.matmul(aps[:], umv, tot_bf[:], start=True, stop=True)

        # within-chunk suffix sums, seeded with the later-chunk total
        # (the scan reads its seed straight from PSUM)
        ss = sp.tile([P, L], F32)
        nc.vector.tensor_tensor_scan(
            ss[:][:, ::-1], em[:][:, ::-1], em[:][:, ::-1],
            initial=aps[:], op0=add, op1=bypass,
        )
        lt = sp.tile([P, L], F32)
        nc.scalar.activation(lt[:], ss[:], ACT.Ln)

        # loss terms: sum over valid groups of (ln(suffix) - xw); the
        # amask offset only lives where wm == 0, so it never contributes
        diff = sp.tile([P, L], F32)
        nc.vector.tensor_sub(diff[:], lt[:], xw[:])
        nc.vector.scalar_tensor_tensor(
            out=diff[:], in0=diff[:], scalar=1.0, in1=wmv,
            op0=bypass, op1=mult, accum_out=res[:, 0:1],
        )
        nc.sync.dma_start(out, res[:], single_packet=True)

    nc.compile()
    return nc


def _host_consts():
    w_idx = np.arange(P)
    um = (
        (w_idx[:, None] // CH == w_idx[None, :] // CH)
        & (w_idx[:, None] % CH > w_idx[None, :] % CH)
    ).astype(np.float32)
    cm = np.ones((P, L), np.float32)
    cm[w_idx % CH == 0, 0] = 0.0
    return um, cm


def make_in_maps(enc, mask, w_fc):
    import ml_dtypes

    bf = ml_dtypes.bfloat16
    um, cm = _host_consts()
    w_bits = np.tile(w_fc[HID:].astype(bf).view(np.uint16)[None, :], (P, 1))
    um_bits = um.astype(bf).view(np.uint16)

    in_maps = []
    for c in range(NCORES):
        mf = mask[c * BS:(c + 1) * BS].reshape(P, L).astype(np.float32)
        wm = mf * cm
        am = (mf - 1.0) * NEGM
        pack = np.empty((P, PK_N), np.uint16)
        pack[:, PK_UM:PK_UM + P] = um_bits
        pack[:, PK_MF:PK_MF + 2 * L] = mf.view(np.uint16)
        pack[:, PK_WM:PK_WM + 2 * L] = wm.view(np.uint16)
        pack[:, PK_AM:PK_AM + 2 * L] = am.view(np.uint16)
        in_maps.append({
            "x": np.ascontiguousarray(enc[c * BS:(c + 1) * BS]),
            "pw": w_bits.view(bf),
            "pk": pack.view(bf),
        })
    return in_maps


def kernel(**inputs) -> np.ndarray:
    enc = np.ascontiguousarray(np.asarray(inputs["encoder_output"], np.float32))
    mask = np.ascontiguousarray(np.asarray(inputs["mask"], np.int32))
    w_fc = np.asarray(inputs["w_fc"], np.float32)

    if "nc" not in _cache:
        _cache["nc"] = _build_nc()
    nc = _cache["nc"]

    res = bass_utils.run_bass_kernel_spmd(
        nc, make_in_maps(enc, mask, w_fc), core_ids=list(range(NCORES))
    )
    o = np.stack([r["out"] for r in res.results]).astype(np.float64)
    num = o[:, :, 0].sum()
    den = o[:, :, 1].sum() - B
    return np.asarray(num / den, dtype=np.float32)
